# revision 33
# baseline (speedup 1.0000x reference)
"""Multi-head attention (b=8, n=1024, dim=1024, 16 heads) on 8 TRN2 NeuronCores.

Data-parallel: one batch element per core. Each core runs an identical
Bass/Tile program computing qkv projection, softmax attention, and the
output projection for its [1024, 1024] slice, in bf16 with fp32 PSUM
accumulation.

Layout choices (host pre-transposes so the device never transposes):
  - xt   [c, n]   = x[i].T                       (bf16)
  - wqkt [c, 2h*d] = permuted q/k weights^T: head-pair p occupies
        f-tiles 2p (q rows of heads 2p,2p+1) and 2p+1 (k rows).
        A 128-row f-tile = [head 2p (64 rows); head 2p+1 (64 rows)], so
        the qkv matmul directly yields q^T/k^T pair tiles where the even
        head lives on partitions 0-63 and the odd head on 64-127.
  - wvt  [c, h*d] = wv.T, wpt [c, o] = w_proj.T  (bf16)

Per core:
  V    = x @ wv^T          -> SBUF [n, h*65] with a ones column per head
  qk^T = wqk_perm @ x^T    -> SBUF pair tiles [128, n]
  S^T  = k_h @ q_h^T       -> PSUM [nk_tile, nq]   (K=64 row-tiled pairs)
  attn^T = exp(0.125*S^T)  -> SBUF bf16 (ScalarE; no max subtraction --
           scores ~ N(0,1), exp stays well inside fp32/bf16 range, and
           softmax is shift-invariant so the result matches jax.nn.softmax)
  out^T_aug = V_aug^T @ attn^T -> PSUM [65, nq]; row 64 = softmax denom
  out^T = out^T_aug[0:64] * (1/denom)  (psum evacuated to SBUF right away
         to free the accumulator banks; reciprocal row replicated across
         partitions with a GPSIMD partition_broadcast; DVE mul)
  y    = out_heads @ w_proj^T + b   in two pieces: contraction chunks
         0-6 run during the last pair's AV phase (bias folded into the
         partial, staged bf16 in the then-dead xt SBUF region); chunk 7
         plus the partial-add form a short tail.

Software pipeline: the pair-0 S/exp prologue and every pair-p step drain
"filler" PE work units (qkv f-tiles, V tiles, proj partials) so the PE
never idles while ACT (exp) is the phase bottleneck. A few warm-up
matmuls on a zeroed tile run during the input DMA wait so the PE HAM
clock-gate is already at full rate when real work arrives.

All inputs are packed into one [dim, 5120] bf16 DRAM tensor so the whole
input loads with 16 large DMAs spread over 4 DGE queues. Output is
stored bf16 (halves the store DMA) and cast to fp32 on host.
"""

import numpy as np
import ml_dtypes

B, N, DIM = 8, 1024, 1024
H, D = 16, 64
NP = 128  # partitions
NCHUNK = 512  # matmul free-dim chunk (one PSUM bank of fp32)
CT = DIM // NP  # 8 contraction chunks
NT = N // NP  # 8 n-tiles
NQC = N // NCHUNK  # 2 nq chunks
PAIRS = H // 2  # 8 head pairs

BF16 = ml_dtypes.bfloat16

_CACHE = {}


def build(loop_iters=1):
    """Build and compile the per-core Bacc graph. Cached per loop_iters."""
    if loop_iters in _CACHE:
        return _CACHE[loop_iters]

    import concourse.mybir as mybir
    import concourse.tile as tile
    from concourse import bacc

    fp32 = mybir.dt.float32
    bf16 = mybir.dt.bfloat16
    Exp = mybir.ActivationFunctionType.Exp

    nc = bacc.Bacc("TRN2", target_bir_lowering=False, debug=False, num_devices=8)

    PACK = N + 2 * DIM + DIM + DIM  # xt | wqkt | wvt | wpt along free dim
    inp = nc.declare_dram_parameter("inp", [DIM, PACK], bf16, isOutput=False)
    bias = nc.declare_dram_parameter("bias", [1, DIM], bf16, isOutput=False)
    out = nc.declare_dram_parameter("out", [N, DIM], bf16, isOutput=True)

    with tile.TileContext(nc) as tc:
        with (
            tc.tile_pool(name="weights", bufs=1) as wpool,
            tc.tile_pool(name="acts", bufs=1) as apool,
            tc.tile_pool(name="attn", bufs=20) as attnpool,
            tc.tile_pool(name="small", bufs=2) as spool,
            tc.tile_pool(name="big_ps", bufs=3, space="PSUM") as big_ps,
            tc.tile_pool(name="half_ps", bufs=2, space="PSUM") as half_ps,
        ):
            # ---- persistent SBUF tensors (loaded once, one DMA per c-chunk) ----
            packed_sb = wpool.tile([NP, CT, PACK], bf16, tag="packed")

            HOT = N + 4 * NP  # xt + wqkt f-tiles of pairs 0 and 1
            dma_engines = [nc.sync, nc.scalar, nc.gpsimd]
            # one DMA per queue per region (3 c-chunks batched) -- fewer,
            # larger DMAs beat per-chunk issue+sem overhead on arrival time
            for ct in range(CT):
                eng = dma_engines[ct % 3]
                eng.dma_start(packed_sb[:, ct, 0:HOT],
                              inp[ct * NP:(ct + 1) * NP, 0:HOT])
            for ct in range(CT):
                eng = dma_engines[ct % 3]
                eng.dma_start(packed_sb[:, ct, HOT:],
                              inp[ct * NP:(ct + 1) * NP, HOT:])
            bias_bc = wpool.tile([NP, DIM], bf16, tag="biasbc")
            nc.sync.dma_start(bias_bc[:], bias[0:1, :].to_broadcast((NP, DIM)))
            xt_sb = packed_sb[:, :, 0:N]
            wqkt_sb = packed_sb[:, :, N:N + 2 * DIM]
            wvt_sb = packed_sb[:, :, N + 2 * DIM:N + 3 * DIM]
            wpt_sb = packed_sb[:, :, N + 3 * DIM:N + 4 * DIM]
            # proj partial-sum staging aliases the xt region: xt's last
            # reader (a qkv f-tile unit) finishes two pairs before the
            # partials are written, and the dep tracker orders the reuse.
            ypart = packed_sb[:, :, 0:N]

            # ---- HAM pre-warm: short back-to-back matmuls on a zeroed
            # tile keep the PE busy during the input-DMA wait so the
            # clock-gate releases before real matmuls start.
            warm = wpool.tile([NP, NCHUNK], bf16, tag="warm")
            nc.vector.memset(warm[:], 0.0)
            warm_ps = big_ps.tile([NP, NCHUNK], fp32, tag="big", name="warm_ps")
            for _ in range(16):
                nc.tensor.matmul(warm_ps[:, 0:NP], lhsT=warm[:, 0:NP],
                                 rhs=warm[:, 0:NP], start=True, stop=True)

            def body(_it=None):
                # ---- per-iteration SBUF ----
                q_sb = apool.tile([NP, PAIRS, N], bf16, tag="q")
                k_sb = apool.tile([NP, PAIRS, N], bf16, tag="k")
                vaug_sb = apool.tile([NP, NT, H * (D + 1)], bf16, tag="vaug")
                outT_sb = apool.tile([NP, CT, N], bf16, tag="outT")

                # ---- filler units: independent PE work drained into the
                # ACT-bound stretches so the PE never starves.
                filler = []

                def drain(steps_left):
                    n = (len(filler) + steps_left - 1) // steps_left
                    for _ in range(n):
                        if filler:
                            filler.pop(0)()

                def emit_qkv_ft(p, which, nqc):
                    # one nq chunk of pair p's q^T (which=0) or k^T (which=1)
                    ft = 2 * p + which
                    dst = q_sb if which == 0 else k_sb
                    qk_ps = big_ps.tile([NP, NCHUNK], fp32, tag="big",
                                        name="qk_ps")
                    for ct in range(CT):
                        nc.tensor.matmul(
                            qk_ps[:],
                            lhsT=wqkt_sb[:, ct, ft * NP:(ft + 1) * NP],
                            rhs=xt_sb[:, ct, nqc * NCHUNK:(nqc + 1) * NCHUNK],
                            start=(ct == 0),
                            stop=(ct == CT - 1),
                        )
                    nc.vector.tensor_copy(
                        dst[:, p, nqc * NCHUNK:(nqc + 1) * NCHUNK], qk_ps[:])

                def emit_v(nt, fc):
                    # heads 8*fc .. 8*fc+7 of V rows nt*128..; ones column
                    # per head appended for the softmax denominator
                    v_ps = big_ps.tile([NP, NCHUNK], fp32, tag="big",
                                       name="v_ps")
                    for ct in range(CT):
                        nc.tensor.matmul(
                            v_ps[:],
                            lhsT=xt_sb[:, ct, nt * NP:(nt + 1) * NP],
                            rhs=wvt_sb[:, ct, fc * NCHUNK:(fc + 1) * NCHUNK],
                            start=(ct == 0),
                            stop=(ct == CT - 1),
                        )
                    vrow = vaug_sb[:, nt, :].rearrange("p (h e) -> p h e",
                                                       e=D + 1)
                    if fc == 0:
                        nc.vector.memset(vrow[:, :, D:D + 1], 1.0)
                    nc.vector.tensor_copy(
                        vrow[:, 8 * fc:8 * fc + 8, 0:D],
                        v_ps[:].rearrange("p (h e) -> p h e", e=D),
                    )

                def emit_proj_partial(nt):
                    # y partial: contraction chunks 0..6, bias folded in,
                    # staged bf16 into the dead xt region
                    for oc in range(NQC):
                        yp = big_ps.tile([NP, NCHUNK], fp32, tag="big",
                                         name="yp")
                        for ct in range(CT - 1):
                            nc.tensor.matmul(
                                yp[:],
                                lhsT=outT_sb[:, ct, nt * NP:(nt + 1) * NP],
                                rhs=wpt_sb[:, ct, oc * NCHUNK:(oc + 1) * NCHUNK],
                                start=(ct == 0),
                                stop=(ct == CT - 2),
                            )
                        nc.vector.tensor_add(
                            ypart[:, nt, oc * NCHUNK:(oc + 1) * NCHUNK], yp[:],
                            bias_bc[:, oc * NCHUNK:(oc + 1) * NCHUNK],
                        )

                def emit_s_exp(p, nkt, atn):
                    # S^T for both heads of pair p at nk-tile nkt; even head
                    # on PE rows 0-63, odd on 64-127 (row-tiled, concurrent)
                    sps = {}
                    for hh in range(2):
                        sps[hh] = big_ps.tile([NP, 2 * NCHUNK], fp32,
                                              tag="big", name="s_ps")
                    for nqc in range(NQC):
                        for hh in range(2):
                            lo, hi = hh * D, (hh + 1) * D
                            nc.tensor.matmul(
                                sps[hh][:, nqc * NCHUNK:(nqc + 1) * NCHUNK],
                                lhsT=k_sb[lo:hi, p, nkt * NP:(nkt + 1) * NP],
                                rhs=q_sb[lo:hi, p, nqc * NCHUNK:(nqc + 1) * NCHUNK],
                                start=True,
                                stop=True,
                                tile_position=(hh * D, 0),
                            )
                    for hh in range(2):
                        a = attnpool.tile([NP, N], bf16, tag="attn")
                        nc.scalar.activation(a[:], sps[hh][:], Exp,
                                             scale=float(D) ** -0.5)
                        atn[hh, nkt] = a

                def emit_av_group(p, hh, nqc):
                    # one softmax-weighted value accumulation [65, 512]:
                    # all 16 exp(p) tiles exist by pair-p start, so each
                    # (nqc, hh) runs as its own accumulation group and its
                    # out-chain overlaps the next group's matmuls
                    h = 2 * p + hh
                    avt = half_ps.tile([D + 1, NCHUNK], fp32, tag="half",
                                       name="av")
                    for nkt in range(NT):
                        nc.tensor.matmul(
                            avt[:],
                            lhsT=vaug_sb[:, nkt, h * (D + 1):(h + 1) * (D + 1)],
                            rhs=atn_cur[hh, nkt][
                                :, nqc * NCHUNK:(nqc + 1) * NCHUNK],
                            start=(nkt == 0),
                            stop=(nkt == NT - 1),
                        )
                    # evacuate right away (frees the accumulator bank),
                    # then scale from SBUF
                    t = spool.tile([D + 1, NCHUNK], bf16, tag="avsb",
                                   bufs=4, name="avsb")
                    nc.vector.tensor_copy(t[:], avt[:])
                    recip = spool.tile([1, NCHUNK], fp32, tag="recip")
                    nc.vector.reciprocal(recip[:], t[D:D + 1, :])
                    recip_b = spool.tile([D, NCHUNK], fp32, tag="recipb")
                    nc.gpsimd.partition_broadcast(recip_b[:], recip[:],
                                                  channels=D)
                    dst = outT_sb[hh * D:(hh + 1) * D, p,
                                  nqc * NCHUNK:(nqc + 1) * NCHUNK]
                    if hh == 0:
                        nc.vector.tensor_mul(dst, t[0:D, :], recip_b[:])
                    else:
                        tmp = spool.tile([D, NCHUNK], bf16, tag="tmpodd")
                        nc.vector.tensor_mul(tmp[:], t[0:D, :], recip_b[:])
                        # partition shift 0:64 -> 64:128 via DMA
                        nc.sync.dma_start(dst, tmp[:])

                def emit_y_tail(nt, eng):
                    y_sb = spool.tile([NP, N], bf16, tag="ysb", bufs=3,
                                      name="y_sb")
                    for oc in range(NQC):
                        yp2 = big_ps.tile([NP, NCHUNK], fp32, tag="big",
                                          name="yp2")
                        nc.tensor.matmul(
                            yp2[:],
                            lhsT=outT_sb[:, CT - 1, nt * NP:(nt + 1) * NP],
                            rhs=wpt_sb[:, CT - 1, oc * NCHUNK:(oc + 1) * NCHUNK],
                            start=True,
                            stop=True,
                        )
                        nc.vector.tensor_add(
                            y_sb[:, oc * NCHUNK:(oc + 1) * NCHUNK], yp2[:],
                            ypart[:, nt, oc * NCHUNK:(oc + 1) * NCHUNK],
                        )
                    eng.dma_start(out[nt * NP:(nt + 1) * NP, :], y_sb[:])

                # ---- schedule ----
                # qkv(0) immediately; the S(0)/exp prologue drains qkv(1)
                # and all V tiles as filler (PE-bound is fine -- ACT has
                # slack); each pair is four AV groups, each preceded by
                # two S(p+1) emissions and filler.
                for which in range(2):
                    for nqc in range(NQC):
                        emit_qkv_ft(0, which, nqc)
                for which in range(2):
                    for nqc in range(NQC):
                        filler.append(
                            lambda w=which, q=nqc: emit_qkv_ft(1, w, q))
                for nt in range(4):
                    for fc in range(2):
                        filler.append(lambda n=nt, f=fc: emit_v(n, f))

                atn_cur = {}
                for nkt in range(NT):
                    emit_s_exp(0, nkt, atn_cur)
                    drain(NT - nkt)

                for p in range(PAIRS):
                    if p + 2 < PAIRS:
                        for which in range(2):
                            for nqc in range(NQC):
                                filler.append(
                                    lambda w=which, q=nqc, pp=p + 2:
                                    emit_qkv_ft(pp, w, q))
                    last = p == PAIRS - 1
                    if last:
                        for nt in range(NT):
                            filler.append(lambda n=nt: emit_proj_partial(n))

                    # hh-major group order frees the even head's attn tiles
                    # mid-pair (bounds the attn pool); pair 7 goes
                    # nqc-major so the first output rows unlock early
                    if last:
                        order = [(q, h) for q in range(NQC) for h in (1, 0)]
                    else:
                        order = [(q, h) for h in range(2) for q in range(NQC)]
                    atn_nxt = {}
                    snkt = 0
                    for g, (nqc, hh) in enumerate(order):
                        if p + 1 < PAIRS:
                            emit_s_exp(p + 1, snkt, atn_nxt)
                            emit_s_exp(p + 1, snkt + 1, atn_nxt)
                            snkt += 2
                        if p == 0 and g == 0:
                            # V tiles 4-7 land just before their first AV
                            # reads; keeping them out of the prologue evens
                            # the PE pressure against the exp pace there
                            for nt in range(4, NT):
                                for fc in range(2):
                                    emit_v(nt, fc)
                        if not last:
                            drain(4 - g)
                        emit_av_group(p, hh, nqc)
                        if last:
                            # partials after the group so they cover the
                            # out-chain; tail rows unlock per nqc half
                            drain(4 - g)
                            if g == 1:
                                for nt in range(4):
                                    emit_y_tail(nt, dma_engines[nt % 3])
                    atn_cur = atn_nxt

                for nt in range(4, NT):
                    emit_y_tail(nt, dma_engines[nt % 3])

            if loop_iters == 1:
                body()
            else:
                with tc.For_i(0, loop_iters, 1) as it:
                    body(it)

    nc.compile()
    _CACHE[loop_iters] = nc
    return nc


def prep_inputs(x, w_qkv, w_proj, b_proj):
    """Host-side sharding + layout prep -> per-core input maps."""
    wq, wk, wv = w_qkv[0:DIM], w_qkv[DIM:2 * DIM], w_qkv[2 * DIM:3 * DIM]
    perm = []
    for p in range(PAIRS):
        perm.append(wq[2 * p * D:(2 * p + 2) * D])
        perm.append(wk[2 * p * D:(2 * p + 2) * D])
    wqk_perm = np.concatenate(perm, axis=0)  # [2*DIM, DIM]
    w_cols = np.concatenate([wqk_perm.T, wv.T, w_proj.T], axis=1).astype(BF16)
    bias = b_proj.reshape(1, DIM).astype(BF16)
    in_maps = []
    for i in range(B):
        xt = x[i].T.astype(BF16)
        inp = np.ascontiguousarray(np.concatenate([xt, w_cols], axis=1))
        in_maps.append({"inp": inp, "bias": bias})
    return in_maps


def kernel(x, w_qkv, w_proj, b_proj):
    from concourse import bass_utils

    x = np.asarray(x, dtype=np.float32)
    w_qkv = np.asarray(w_qkv, dtype=np.float32)
    w_proj = np.asarray(w_proj, dtype=np.float32)
    b_proj = np.asarray(b_proj, dtype=np.float32)
    assert x.shape == (B, N, DIM)

    nc = build(1)
    in_maps = prep_inputs(x, w_qkv, w_proj, b_proj)
    res = bass_utils.run_bass_kernel_spmd(nc, in_maps, core_ids=list(range(B)))
    return np.stack(
        [res.results[i]["out"].astype(np.float32) for i in range(B)], axis=0)
